# revision 14
# baseline (speedup 1.0000x reference)
"""LoRA layer kernel for Trainium2 (8 NeuronCores, data-parallel).

Computes out = SCALING * (x @ A^T) @ B^T for x [4, 8192, 1024],
lora_A [4, 1024], lora_B [1024, 4], SCALING = 0.25.

Strategy (per core, shard = 4096 rows x 1024 features), fp16 end-to-end
(rel err ~6e-4 vs the 2e-2 gate; halves HBM traffic vs f32):
  - Host casts x to fp16 and pre-transposes each core's shard so feature
    chunks sit on partitions -> no PE transposes on device.
  - The whole 8 MiB x shard stays RESIDENT in SBUF (64 KiB/partition);
    all 8 1-MiB loads are issued up-front, split between the sync and
    gpsimd DMA rings, so the load stream is never gated by compute.
  - Uniform serial matmuls (every matmul at array position (0,0), same
    shapes throughout): the PE HAM clock-gate releases to full 2.4 GHz
    only under a sustained uniform instruction stream -- measured 216 ns
    per 512-col matmul vs 512 ns for array-packed variants whose mode
    thrash keeps the clock at 1 GHz. Serial-at-2.4GHz beats packed-at-1.
  - Per 512-row tile: 8 accumulating stage-1 matmuls (contraction 128
    per feature chunk) -> ht PSUM bank, DVE cast-copy to fp16, 8
    stage-2 matmuls (rank-4 contraction, rows back on partitions),
    ScalarE/DVE alternate PSUM->SBUF fp16 evacuation, one flat 1 MiB
    store per tile on the scalar HWDGE ring.
  - Host upcasts the fp16 result to f32 and un-permutes.
"""

import sys

for _p in (
    "/root/.axon_site",
    "/root/.axon_site/_ro/trn_rl_repo",
    "/root/.axon_site/_ro/pypackages",
):
    if _p not in sys.path:
        sys.path.insert(0, _p)

from contextlib import ExitStack

import numpy as np

N_CORES = 8
D_IN = 1024
D_OUT = 1024
RANK = 4
ROWS_TOTAL = 4 * 8192
ROWS_PER_CORE = ROWS_TOTAL // N_CORES  # 4096
SCALING = 1.0 / RANK

P = 128               # partitions
N_CHUNKS = D_IN // P  # 8 feature chunks of 128
G = 512               # rows per tile (one PSUM accumulation chain)
J = G // P            # 4 row subtiles per tile

PLAN = [(i * G, 1) for i in range(ROWS_PER_CORE // G)]  # 8 tiles of 512 rows
N_XBLOCKS = ROWS_PER_CORE // G * N_CHUNKS  # 64 x-blocks of [128, 512]
OUT_WORDS = ROWS_PER_CORE // P * D_OUT     # 32768 fp16 words per partition


def emit_lora(tc, xt_ap, at_ap, bt_ap, out_ap):
    """Emit the LoRA kernel IR for one core's 4096-row shard.

    xt_ap : DRAM [P, 64, 512] fp16, xt[p, t*8 + c, m] =
            x[t*512 + m, c*128 + p]  (pre-transposed by host)
    at_ap : DRAM [P, N_CHUNKS, RANK] fp16, at[p, c, r] = A[r, c*128 + p]
    bt_ap : DRAM [P, D_OUT] fp16, bt[r, o] = SCALING * B[o, r] on
            partitions 0-3
    out_ap: DRAM [P, 32768] fp16, out[p, t*4096 + j*1024 + o2*512 + m] =
            result[t*512 + j*128 + p, o2*512 + m]
    """
    import concourse.mybir as mybir

    nc = tc.nc
    f32 = mybir.dt.float32
    f16 = mybir.dt.float16
    ctx = tc._ctx  # ExitStack owned by caller

    consts = ctx.enter_context(tc.tile_pool(name="consts", bufs=1))
    htpool = ctx.enter_context(tc.tile_pool(name="ht", bufs=3))
    opool = ctx.enter_context(tc.tile_pool(name="osb", bufs=4))
    ps_ht = ctx.enter_context(tc.tile_pool(name="ps_ht", bufs=2, space="PSUM"))
    ps_o = ctx.enter_context(tc.tile_pool(name="ps_o", bufs=5, space="PSUM"))

    # Tiny constants lead the scalar (ACT) HWDGE ring, which is otherwise
    # idle at kernel start.
    at_sb = consts.tile([P, N_CHUNKS, RANK], f16)
    nc.scalar.dma_start(at_sb[:], at_ap[:])
    bt_sb = consts.tile([P, D_OUT], f16)
    nc.scalar.dma_start(bt_sb[:], bt_ap[:])

    # The whole 8 MiB x shard fits in SBUF: keep it resident and stream
    # ALL loads up-front with no slot rotation. Alternating tiles ride
    # the sync (HWDGE) and gpsimd (SWDGE) rings so two independent load
    # queues feed the 16 SDMA engines.
    x_all = consts.tile([P, N_XBLOCKS, G], f16)
    for t in range(len(PLAN)):
        src = xt_ap[:, t * N_CHUNKS : (t + 1) * N_CHUNKS, :]
        dst = x_all[:, t * N_CHUNKS : (t + 1) * N_CHUNKS, :]
        if t % 2 == 0:
            nc.sync.dma_start(dst, src)
        else:
            nc.gpsimd.dma_start(dst, src)

    evac_ctr = 0
    for row0, _ in PLAN:
        t = row0 // G
        w0 = row0 * 8  # first output word (per partition)

        # Stage 1: ht[r, m] = sum_f A[r, f] x[row m, f], accumulated over
        # the 8 feature chunks into one PSUM bank.
        ht_ps = ps_ht.tile([RANK, G], f32, name="ht_ps")
        for c in range(N_CHUNKS):
            nc.tensor.matmul(
                ht_ps[:],
                lhsT=at_sb[:, c, :],
                rhs=x_all[:, t * N_CHUNKS + c, :],
                start=(c == 0),
                stop=(c == N_CHUNKS - 1),
            )
        ht_sb = htpool.tile([RANK, G], f16, name="ht_sb")
        nc.vector.tensor_copy(ht_sb[:], ht_ps[:])

        # Stage 2: out[m, o] = sum_r ht[r, m] bt[r, o]; rows back on
        # partitions so the store is contiguous. ScalarE/DVE alternate on
        # PSUM->SBUF fp16 evacuation.
        o_sb = opool.tile([P, J * D_OUT], f16, name="o_sb")
        for j in range(J):
            for o2 in range(2):
                o_ps = ps_o.tile([P, 512], f32, name="o_ps")
                nc.tensor.matmul(
                    o_ps[:],
                    lhsT=ht_sb[:, j * P : (j + 1) * P],
                    rhs=bt_sb[:RANK, o2 * 512 : (o2 + 1) * 512],
                    start=True,
                    stop=True,
                )
                off = (j * 2 + o2) * 512
                tgt = o_sb[:, off : off + 512]
                if evac_ctr % 2 == 0:
                    nc.scalar.copy(tgt, o_ps[:])
                else:
                    nc.vector.tensor_copy(tgt, o_ps[:])
                evac_ctr += 1

        # Stores ride the second HWDGE ring (ACT), never head-of-line
        # blocking the sync ring carrying the loads.
        nc.scalar.dma_start(out_ap[:, w0 : w0 + J * D_OUT], o_sb[:])


def build_nc():
    import concourse.mybir as mybir
    import concourse.tile as tile
    from concourse import bacc

    f16 = mybir.dt.float16
    nc = bacc.Bacc("TRN2", target_bir_lowering=False, debug=False)
    xt_d = nc.dram_tensor("xt", [P, N_XBLOCKS, G], f16, kind="ExternalInput").ap()
    at_d = nc.dram_tensor("at", [P, N_CHUNKS, RANK], f16, kind="ExternalInput").ap()
    bt_d = nc.dram_tensor("bt", [P, D_OUT], f16, kind="ExternalInput").ap()
    out_d = nc.dram_tensor(
        "out", [P, OUT_WORDS], f16, kind="ExternalOutput"
    ).ap()

    with tile.TileContext(nc) as tc:
        with ExitStack() as ctx:
            tc._ctx = ctx
            emit_lora(tc, xt_d, at_d, bt_d, out_d)
    nc.compile()
    return nc


def host_prep(lora_A, lora_B):
    # at[p, c, r] = A[r, c*128 + p]
    at = np.ascontiguousarray(
        np.asarray(lora_A, dtype=np.float32)
        .reshape(RANK, N_CHUNKS, P)
        .transpose(2, 1, 0)
    ).astype(np.float16)
    # bt[r, o] = SCALING * B[o, r] on partitions 0-3 (zero-padded to 128)
    btv = (np.asarray(lora_B, dtype=np.float32).T * SCALING).astype(np.float16)
    bt = np.zeros((P, D_OUT), dtype=np.float16)
    bt[:RANK] = btv
    return at, bt


def stage_x(x):
    """x [4, 8192, 1024] f32 -> per-core [P, N_XBLOCKS, 512] fp16 shards."""
    xh = np.asarray(x, dtype=np.float32).reshape(
        N_CORES, ROWS_PER_CORE // G, G, N_CHUNKS, P
    )
    # (core, t, m, c, p) -> (core, p, t, c, m)
    xs = np.ascontiguousarray(xh.transpose(0, 4, 1, 3, 2)).astype(np.float16)
    return xs.reshape(N_CORES, P, N_XBLOCKS, G)


def unstage_out(res_list):
    """Per-core [P, 32768] fp16 -> out [4, 8192, 1024] f32."""
    o = np.stack(res_list, axis=0).astype(np.float32)
    blk = o.reshape(N_CORES, P, ROWS_PER_CORE // G, J, D_OUT)
    # (core, p, t, j, o) -> (core, t, j, p, o)
    out = blk.transpose(0, 2, 3, 1, 4).reshape(N_CORES, ROWS_PER_CORE, D_OUT)
    return np.ascontiguousarray(out).reshape(4, 8192, D_OUT)


_NC_CACHE = {}


def kernel(x, lora_A, lora_B):
    from concourse.bass_utils import run_bass_kernel_spmd

    if "nc" not in _NC_CACHE:
        _NC_CACHE["nc"] = build_nc()
    nc = _NC_CACHE["nc"]

    xs = stage_x(x)
    at, bt = host_prep(lora_A, lora_B)
    in_maps = [
        {"xt": np.ascontiguousarray(xs[i]), "at": at, "bt": bt}
        for i in range(N_CORES)
    ]
    res = run_bass_kernel_spmd(nc, in_maps, core_ids=list(range(N_CORES)))
    return unstage_out([res.results[i]["out"] for i in range(N_CORES)])


# revision 15
# speedup vs baseline: 1.2906x; 1.2906x over previous
"""LoRA layer kernel for Trainium2 (8 NeuronCores, data-parallel).

Computes out = SCALING * (x @ A^T) @ B^T for x [4, 8192, 1024],
lora_A [4, 1024], lora_B [1024, 4], SCALING = 0.25.

Strategy (per core, shard = 4096 rows x 1024 features), fp16 end-to-end
(rel err ~6e-4 vs the 2e-2 gate; halves HBM traffic vs f32):
  - Host casts x to fp16 and pre-transposes each core's shard so feature
    chunks sit on partitions -> no PE transposes on device.
  - The whole 8 MiB x shard stays RESIDENT in SBUF (64 KiB/partition);
    all loads are issued up-front, split between the sync and gpsimd DMA
    rings, so the load stream is never gated by compute.
  - PE array packing (tile_position): stage 1 column-tiled 128x32 (ht of
    group q at PSUM partitions 32q.. of one shared bank, chains run
    concurrently), stage 2 row-tiled 32x128 in group PAIRS: two groups'
    matmuls fill the two banks of one PSUM tile and one wide [128, 1024]
    copy evacuates both (ScalarE/DVE alternate), halving evacuation
    instruction count.
  - Stores are j-slice granular (one store per 128-row slice across the
    sg's groups) so the store stream flows smoothly instead of bunching
    at super-group boundaries.
  - Warm-up matmuls during the DMA fill nudge the HAM clock-gate.
  - Tapered super-groups [512, 2048, 1024, 512] rows.
  - Host upcasts the fp16 result to f32 and un-permutes.
"""

import sys

for _p in (
    "/root/.axon_site",
    "/root/.axon_site/_ro/trn_rl_repo",
    "/root/.axon_site/_ro/pypackages",
):
    if _p not in sys.path:
        sys.path.insert(0, _p)

from contextlib import ExitStack

import numpy as np

N_CORES = 8
D_IN = 1024
D_OUT = 1024
RANK = 4
ROWS_TOTAL = 4 * 8192
ROWS_PER_CORE = ROWS_TOTAL // N_CORES  # 4096
SCALING = 1.0 / RANK

P = 128               # partitions
N_CHUNKS = D_IN // P  # 8 feature chunks of 128
G = 512               # rows per group (one PSUM accumulation chain)
J = G // P            # 4 row subtiles per group

# Super-group plan: (row0, n_groups). Each sg runs n_groups 512-row groups
# concurrently in disjoint PE array tiles (partition stride 128//n_groups).
PLAN = [(0, 1), (512, 4), (2560, 2), (3584, 1)]
N_XBLOCKS = ROWS_PER_CORE // G * N_CHUNKS  # 64 x-blocks of [128, 512]
OUT_WORDS = ROWS_PER_CORE // P * D_OUT     # 32768 fp16 words per partition
N_WARMUP = 10


def emit_lora(tc, xt_ap, at_ap, bt_ap, out_ap):
    """Emit the LoRA kernel IR for one core's 4096-row shard.

    xt_ap : DRAM [P, 64, 512] fp16; block b = sg-major [c][g][m]:
            xt[p, (row0//512)*8 + c*n_g + g, m] = x[row0 + g*512 + m, c*128 + p]
    at_ap : DRAM [P, N_CHUNKS, RANK] fp16, at[p, c, r] = A[r, c*128 + p]
    bt_ap : DRAM [P, D_OUT] fp16, bt[32k + r, o] = SCALING * B[o, r] (k<4),
            zero elsewhere (replicated for row-tiled stage 2)
    out_ap: DRAM [P, 32768] fp16; per sg, [j][o2][q][m] blocks:
            out[p, row0*8 + ((j*2 + o2)*n_g + q)*512 + m] =
            result[row0 + q*512 + j*128 + p, o2*512 + m]
    """
    import concourse.mybir as mybir

    nc = tc.nc
    f32 = mybir.dt.float32
    f16 = mybir.dt.float16
    ctx = tc._ctx  # ExitStack owned by caller

    consts = ctx.enter_context(tc.tile_pool(name="consts", bufs=1))
    htpool = ctx.enter_context(tc.tile_pool(name="ht", bufs=3))
    opool = ctx.enter_context(tc.tile_pool(name="osb", bufs=2))
    ps_ht = ctx.enter_context(tc.tile_pool(name="ps_ht", bufs=2, space="PSUM"))
    ps_o = ctx.enter_context(tc.tile_pool(name="ps_o", bufs=3, space="PSUM"))

    # Tiny constants lead the scalar (ACT) HWDGE ring, which is otherwise
    # idle at kernel start.
    at_sb = consts.tile([P, N_CHUNKS, RANK], f16)
    nc.scalar.dma_start(at_sb[:], at_ap[:])
    bt_sb = consts.tile([P, D_OUT], f16)
    nc.scalar.dma_start(bt_sb[:], bt_ap[:])

    # The whole 8 MiB x shard fits in SBUF: keep it resident and stream
    # ALL loads up-front with no slot rotation. Alternating blocks ride
    # the sync (HWDGE) and gpsimd (SWDGE) rings so two independent load
    # queues feed the 16 SDMA engines.
    x_all = consts.tile([P, N_XBLOCKS, G], f16)
    i = 0
    for row0, n_g in PLAN:
        xb = row0 // G * N_CHUNKS
        for c in range(N_CHUNKS):
            src = xt_ap[:, xb + c * n_g : xb + (c + 1) * n_g, :]
            dst = x_all[:, xb + c * n_g : xb + (c + 1) * n_g, :]
            if i % 2 == 0:
                nc.sync.dma_start(dst, src)
            else:
                nc.gpsimd.dma_start(dst, src)
            i += 1

    # Warm-up: matmuls on a zeroed tile keep the PE active during the DMA
    # fill so the HAM clock-gate releases before the first real matmul.
    zt = consts.tile([P, G], f16)
    nc.vector.memzero(zt[:])
    for w in range(N_WARMUP):
        w_ps = ps_o.tile([P, G], f32, name="o_ps")
        nc.tensor.matmul(
            w_ps[:], lhsT=zt[:, :P], rhs=zt[:], start=True, stop=True
        )

    evac_ctr = 0
    for row0, n_g in PLAN:
        stride = P // n_g
        xb = row0 // G * N_CHUNKS  # first x-block of this sg
        w0 = row0 * 8              # first output word (per partition)

        # Stage 1, column-tiled 128x32: group q's chain accumulates into
        # PSUM partitions q*stride..+3 of one shared bank. Round-robin over
        # q inside each chunk round so the n_g array tiles run concurrently.
        ht_ps = ps_ht.tile([P, G], f32, name="ht_ps")
        for c in range(N_CHUNKS):
            for q in range(n_g):
                nc.tensor.matmul(
                    ht_ps[q * stride : q * stride + RANK, :],
                    lhsT=at_sb[:, c, :],
                    rhs=x_all[:, xb + c * n_g + q, :],
                    start=(c == 0),
                    stop=(c == N_CHUNKS - 1),
                    tile_position=(0, q * stride),
                    skip_group_check=True,
                )
        ht_sb = htpool.tile([P, G], f16, name="ht_sb")
        nc.vector.tensor_copy(ht_sb[:], ht_ps[:])

        # Stage 2, row-tiled 32x128, group-PAIRED: groups q and q+1 fill
        # the two banks of one PSUM tile concurrently; one wide copy
        # evacuates both. After each j-slice completes, its store fires,
        # so stores flow smoothly through the sg instead of bunching.
        o_sb = opool.tile([P, n_g * J * D_OUT], f16, name="o_sb")
        jw = 2 * n_g * 512  # output words per j-slice
        for j in range(J):
            for o2 in range(2):
                for qp in range(0, n_g, 2):
                    npair = min(2, n_g - qp)
                    o_ps = ps_o.tile([P, npair * 512], f32, name="o_ps")
                    for u in range(npair):
                        q = qp + u
                        nc.tensor.matmul(
                            o_ps[:, u * 512 : (u + 1) * 512],
                            lhsT=ht_sb[
                                q * stride : q * stride + RANK,
                                j * P : (j + 1) * P,
                            ],
                            rhs=bt_sb[
                                q * stride : q * stride + RANK,
                                o2 * 512 : (o2 + 1) * 512,
                            ],
                            start=True,
                            stop=True,
                            tile_position=(q * stride, 0),
                        )
                    off = ((j * 2 + o2) * n_g + qp) * 512
                    tgt = o_sb[:, off : off + npair * 512]
                    if evac_ctr % 2 == 0:
                        nc.scalar.copy(tgt, o_ps[:])
                    else:
                        nc.vector.tensor_copy(tgt, o_ps[:])
                    evac_ctr += 1
            # Store this j-slice (all groups, both halves) on the ACT ring.
            nc.scalar.dma_start(
                out_ap[:, w0 + j * jw : w0 + (j + 1) * jw],
                o_sb[:, j * jw : (j + 1) * jw],
            )


def build_nc():
    import concourse.mybir as mybir
    import concourse.tile as tile
    from concourse import bacc

    f16 = mybir.dt.float16
    nc = bacc.Bacc("TRN2", target_bir_lowering=False, debug=False)
    xt_d = nc.dram_tensor("xt", [P, N_XBLOCKS, G], f16, kind="ExternalInput").ap()
    at_d = nc.dram_tensor("at", [P, N_CHUNKS, RANK], f16, kind="ExternalInput").ap()
    bt_d = nc.dram_tensor("bt", [P, D_OUT], f16, kind="ExternalInput").ap()
    out_d = nc.dram_tensor(
        "out", [P, OUT_WORDS], f16, kind="ExternalOutput"
    ).ap()

    with tile.TileContext(nc) as tc:
        with ExitStack() as ctx:
            tc._ctx = ctx
            emit_lora(tc, xt_d, at_d, bt_d, out_d)
    nc.compile()
    return nc


def host_prep(lora_A, lora_B):
    # at[p, c, r] = A[r, c*128 + p]
    at = np.ascontiguousarray(
        np.asarray(lora_A, dtype=np.float32)
        .reshape(RANK, N_CHUNKS, P)
        .transpose(2, 1, 0)
    ).astype(np.float16)
    # bt[32k + r, o] = SCALING * B[o, r], replicated at partition stride 32
    btv = (np.asarray(lora_B, dtype=np.float32).T * SCALING).astype(np.float16)
    bt = np.zeros((P, D_OUT), dtype=np.float16)
    for k in range(4):
        bt[32 * k : 32 * k + RANK] = btv
    return at, bt


def stage_x(x):
    """x [4, 8192, 1024] f32 -> per-core [P, N_XBLOCKS, 512] fp16 shards."""
    xc = np.asarray(x, dtype=np.float32).reshape(N_CORES, ROWS_PER_CORE, D_IN)
    blocks = []
    for row0, n_g in PLAN:
        rows = n_g * G
        xh = xc[:, row0 : row0 + rows].reshape(N_CORES, n_g, G, N_CHUNKS, P)
        # (core, g, m, c, p) -> (core, p, c, g, m)
        blocks.append(xh.transpose(0, 4, 3, 1, 2).reshape(N_CORES, P, -1))
    xs = np.concatenate(blocks, axis=2).astype(np.float16)
    return np.ascontiguousarray(xs.reshape(N_CORES, P, N_XBLOCKS, G))


def unstage_out(res_list):
    """Per-core [P, 32768] fp16 -> out [4, 8192, 1024] f32."""
    o = np.stack(res_list, axis=0).astype(np.float32)
    out = np.empty((N_CORES, ROWS_PER_CORE, D_OUT), dtype=np.float32)
    for row0, n_g in PLAN:
        w0 = row0 * 8
        blk = o[:, :, w0 : w0 + n_g * J * D_OUT].reshape(
            N_CORES, P, J, 2, n_g, 512
        )
        # (core, p, j, o2, q, m) -> (core, q, j, p, o2, m)
        perm = blk.transpose(0, 4, 2, 1, 3, 5).reshape(
            N_CORES, n_g * G, D_OUT
        )
        out[:, row0 : row0 + n_g * G] = perm
    return np.ascontiguousarray(out).reshape(4, 8192, D_OUT)


_NC_CACHE = {}


def kernel(x, lora_A, lora_B):
    from concourse.bass_utils import run_bass_kernel_spmd

    if "nc" not in _NC_CACHE:
        _NC_CACHE["nc"] = build_nc()
    nc = _NC_CACHE["nc"]

    xs = stage_x(x)
    at, bt = host_prep(lora_A, lora_B)
    in_maps = [
        {"xt": np.ascontiguousarray(xs[i]), "at": at, "bt": bt}
        for i in range(N_CORES)
    ]
    res = run_bass_kernel_spmd(nc, in_maps, core_ids=list(range(N_CORES)))
    return unstage_out([res.results[i]["out"] for i in range(N_CORES)])
